# revision 11
# baseline (speedup 1.0000x reference)
"""TRN2 Bass kernel for nn_COV_75359496176097.

reference():
    B2 = B[0]                               # (8192, 8192)
    rn = sqrt(1 / sum(B2*B2, axis=1))       # row norms
    A  = rn * B2 * exp(tile(logstd, 64))[:, None]
    samples = tile(mu,64) + einsum('mk,bk->bm', A, eps[:,:,0])
    returns (mu_out, logvar, samples), each (128, 64, 128)

Strategy: shard A by rows across 8 cores (1024 rows each, no
collectives).  The row-norm and exp(logstd) scalings are diagonal, so
they are folded into A on the host, and the device runs a pure GEMM
out[b, r] = sum_k eps[k, b] * A[r, k], DMA-bound at the per-core HBM
roofline (~358 GB/s; the 8 cores together saturate the chip's HBM).
Bytes are the binding constraint, so A streams in two precision tiers:

  * the N_FP16 rows with the largest exp(logstd)  -> fp16
  * all other rows -> fp8 E3M4 (TRN FP8_EXP3, IEEE bias 3), scaled by
    a global power-of-two C so values sit in fp8's normal range.

The harness error metric is relative to the GLOBAL max |sample|, set
by the largest-exp(logstd) rows; a row whose exp(logstd) is t times
smaller contributes its ~2% fp8 row-relative error only as ~2%/t
globally.  With logstd ~ N(0,1), keeping the top 8 of 1024 rows in
fp16 leaves the worst fp8 row well below the max -> measured 5.3e-3
global error (gate is 2e-2).  eps stays fp16 (its error feeds every
output at full scale).  mu is added by a K=1 matmul (stationary = a
length-1 column of ones) from a tiny fp16 vector, pre-scaled by C on
the fp8 columns.

PSUM start=True clears has_written at BANK granularity (512 fp32
cols), so the fp16 and fp8 accumulation groups must not share a bank:
fp16 accumulates in psum cols [0, n1), the fp8 group at a gap, cols
[512, 512+n2).  The epilogue maps psum cols back to packed output
cols.

The whole working set fits SBUF, so there is NO slot recycling: the 64
k-tile DMAs alternate between both HWDGE queues, throttled only by a
PFD-deep PE-progress window (a core that wins HBM arbitration stops
pulling ahead, letting starved neighbor cores catch up — the max core
defines the score).  Each tile has its OWN completion semaphore — a
shared counter at 16*n is ambiguous (SDMA engines interleave work from
multiple queued DMAs; the shared-counter version was observed to
race).  The PE consumes tiles in order (eps k-slice stationary fp16, A
k-slice moving fp16/fp8 per segment, PSUM-accumulated).  Epilogue: DVE
writes the packed outputs to fp16 SBUF in 4 quarter-chunks (copy for
fp16 columns, *1/C for fp8 columns) and the two queues DMA the
quarters out.  The host un-permutes the row ordering after gathering.

Each k-tile is one DMA of a host-packed byte row:
  [fp16 A block | fp8 A block (padded even) | fp16 eps block]
"""

import sys
from contextlib import ExitStack

if "/opt/trn_rl_repo" not in sys.path:
    sys.path.insert(0, "/opt/trn_rl_repo")

import ml_dtypes
import numpy as np

import concourse.bacc as bacc
import concourse.mybir as mybir
from concourse import bass_utils

Z = 128
NS = 64
M = Z * NS          # 8192
BATCH = 128
NCORES = 8
RPC = M // NCORES   # 1024 rows of A per core
KT = M // 128       # 64 k-tiles
EPSB = 2 * BATCH    # eps block bytes per tile row

N_FP16 = 8          # rows per core kept in fp16 (largest exp(logstd))
GRP = 2             # k-tiles per DMA (contiguous per-partition bytes ->
                    # bigger descriptors -> better wire efficiency)
WARM_MM = 6         # warmup matmuls of N=512 (~2.6us at the cold 1.2GHz
                    # clock): bridge the PE to the first data tiles; the
                    # HAM gate flips to 2.4GHz ~3.4us into the sustained
                    # matmul stream either way
C_FP8 = 32.0        # global fp8 scale (power of two; exact in fp16/fp32)
FP8_CLIP = 15.0     # e3m4 max normal is 15.5
P8 = 512            # psum col where the fp8 accumulation group starts

F8NP = np.dtype(ml_dtypes.float8_e3m4)

f32 = mybir.dt.float32
f16 = mybir.dt.float16
f8 = mybir.dt.float8e3

_nc_cache = {}


def _segments(n1, n2):
    """Matmul segments (psum_a, psum_b, is_fp16): fp16 rows accumulate in
    psum [0, n1), fp8 rows in [P8, P8+n2) so the two accumulation groups
    never share a 512-col psum bank; each segment stays within one bank."""
    assert 0 < n1 <= P8
    segs = [(0, n1, True)]
    for a in range(P8, P8 + n2, 512):
        segs.append((a, min(a + 512, P8 + n2), False))
    return segs


def _pcol(x, n1):
    """packed output col -> psum col"""
    return x if x < n1 else P8 + (x - n1)


def _quarters(n1, n2, segs):
    """For each output quarter [256j, 256j+256): the list of
    (out_a, out_b, psum_a, is_fp16) pieces and the s_acc threshold
    (1 + max index of any segment the quarter reads)."""
    qinfo = []
    for j in range(4):
        qa, qb = j * 256, (j + 1) * 256
        pieces = []
        for a, b in ((qa, min(n1, qb)), (max(n1, qa), qb)):
            if a < b:
                pieces.append((a, b, _pcol(a, n1), b <= n1))
        pieces = list(dict.fromkeys(pieces))
        th = 0
        for a, b, pa, _ in pieces:
            pb = pa + (b - a)
            for i, (sa, sb, _) in enumerate(segs):
                if sa < pb and pa < sb:
                    th = max(th, i + 1)
        qinfo.append((pieces, th))
    return qinfo


def _build(n1, n2):
    n2p = n2 + (n2 & 1)
    wb = 2 * n1 + n2p + EPSB      # packed bytes per tile row
    eps_off = 2 * n1 + n2p
    np_cols = P8 + n2             # psum cols used
    segs = _segments(n1, n2)
    qinfo = _quarters(n1, n2, segs)

    ng = KT // GRP                # DMA groups
    gwb = GRP * wb                # bytes per partition per group

    nc = bacc.Bacc("TRN2", debug=False)

    bte_d = nc.dram_tensor("bte", (ng * 128, gwb), mybir.dt.uint8,
                           kind="ExternalInput")
    mu_d = nc.dram_tensor("mu", (1, np_cols), f16, kind="ExternalInput")
    out_d = nc.dram_tensor("out", (BATCH, RPC), f16, kind="ExternalOutput")

    with ExitStack() as ctx:
        e = ctx.enter_context
        big8 = e(nc.sbuf_tensor("big8", [128, KT * wb], mybir.dt.uint8))
        ones = e(nc.sbuf_tensor("ones", [128, 128], f16))
        wmv = e(nc.sbuf_tensor("wmv", [128, 512], f16))
        mu_sb = e(nc.sbuf_tensor("mu_sb", [1, np_cols], f16))
        out_sb = e(nc.sbuf_tensor("out_sb", [128, RPC], f16))
        acc = e(nc.psum_tensor([128, 1536], f32))
        warm_ps = e(nc.psum_tensor([128, 512], f32))

        # one completion sem per DMA group: sem == 16 requires every one of
        # the 16 SDMA engines to have retired THIS group's descriptors
        s_t = [e(nc.semaphore(name=f"s_t{g}")) for g in range(ng)]
        s_cst = e(nc.semaphore(name="s_cst"))
        s_wm = e(nc.semaphore(name="s_wm"))
        s_acc = e(nc.semaphore(name="s_acc"))
        s_out = e(nc.semaphore(name="s_out"))
        s_out2 = e(nc.semaphore(name="s_out2"))
        s_od = e(nc.semaphore(name="s_od"))

        block = e(nc.Block())

        def rhs_ap(t, sa, sb, is16):
            if is16:
                return big8[:, t * wb + 2 * sa:t * wb + 2 * sb].bitcast(f16)
            off = t * wb + 2 * n1 + (sa - P8)
            return big8[:, off:off + (sb - sa)].bitcast(f8)

        def issue(eng, g):
            # no issue throttle: the whole working set has dedicated SBUF,
            # and HBM arbitration round-robins across cores at packet
            # granularity, so an unpaced stream keeps the DMA engines fed
            # continuously (a PE-progress throttle was measured to feed a
            # stall back into the DMA stream and starve the PE)
            eng.dma_start(
                big8[:, g * gwb:(g + 1) * gwb],
                bte_d.ap()[g * 128:(g + 1) * 128, :],
            ).then_inc(s_t[g], 16)

        @block.sync
        def _(sync):
            for g in range(0, ng, 2):
                issue(sync, g)
            sync.wait_ge(s_out, 1)
            sync.dma_start(out_d.ap()[:, 0:256], out_sb[:, 0:256]).then_inc(
                s_od, 16
            )
            sync.wait_ge(s_out, 3)
            sync.dma_start(out_d.ap()[:, 512:768], out_sb[:, 512:768]).then_inc(
                s_od, 16
            )

        @block.scalar
        def _(scalar):
            scalar.dma_start(mu_sb[:], mu_d.ap()[:, :]).then_inc(s_cst, 16)
            for g in range(1, ng, 2):
                issue(scalar, g)
            scalar.wait_ge(s_out, 2)
            scalar.dma_start(
                out_d.ap()[:, 256:512], out_sb[:, 256:512]
            ).then_inc(s_od, 16)
            scalar.wait_ge(s_out, 4)
            scalar.dma_start(
                out_d.ap()[:, 768:1024], out_sb[:, 768:1024]
            ).then_inc(s_od, 16)
            scalar.wait_ge(s_od, 64)
            scalar.nop()

        @block.tensor
        def _(tensor):
            # sustained warmup (~4.3us of N=512 matmuls at the cold 1.2GHz
            # clock) so the PE HAM clock monitor flips to full speed while
            # the first data tiles are still in flight; the 8-MM version
            # was too short and the whole first ~10us of GEMM ran at half
            # clock
            tensor.wait_ge(s_wm, 1)
            for _ in range(WARM_MM):
                nc.tensor.matmul(
                    warm_ps[:, 0:512], ones[:], wmv[:], start=True, stop=True
                )
            for g in range(KT // GRP):
                tensor.wait_ge(s_t[g], 16)
                for t in range(g * GRP, (g + 1) * GRP):
                    st, sp = t == 0, t == KT - 1
                    eps_v = big8[:, t * wb + eps_off:(t + 1) * wb].bitcast(f16)
                    for si, (sa, sb, is16) in enumerate(segs):
                        ins = nc.tensor.matmul(
                            acc[:, sa:sb], eps_v, rhs_ap(t, sa, sb, is16),
                            start=st, stop=sp,
                        )
                        if sp:
                            ins.then_inc(s_acc, 1)
                    if st:
                        # mu via K=1 matmul: out[b, r] += 1 * mu[r].  Order
                        # within a psum accumulation group doesn't matter,
                        # so run it early (off the critical tail).
                        tensor.wait_ge(s_cst, 16)
                        for sa, sb, _ in segs:
                            nc.tensor.matmul(
                                acc[:, sa:sb], ones[0:1, 0:128],
                                mu_sb[0:1, sa:sb], start=False, stop=False,
                            )

        @block.vector
        def _(vector):
            nc.vector.memset(ones[:], 1.0)
            nc.vector.memset(wmv[:], 1.0).then_inc(s_wm, 1)
            for j in range(4):
                pieces, th = qinfo[j]
                vector.wait_ge(s_acc, th)
                for a, b, pa, is16 in pieces:
                    pb = pa + (b - a)
                    if is16:
                        ins = nc.vector.tensor_copy(
                            out_sb[:, a:b], acc[:, pa:pb]
                        )
                    else:
                        ins = nc.vector.tensor_scalar_mul(
                            out_sb[:, a:b], acc[:, pa:pb], 1.0 / C_FP8
                        )
                ins.then_inc(s_out, 1)

    nc.compile()
    return nc


def _get_nc(n1, n2):
    key = (n1, n2)
    if key not in _nc_cache:
        _nc_cache[key] = _build(n1, n2)
    return _nc_cache[key]


def _prep_inputs(mu, logstd, B, eps):
    B2 = B[0]                                            # (M, M) fp32
    logstd_rep = np.tile(logstd, NS).astype(np.float32)  # (M,)
    mu_rep = np.tile(mu[0], NS).astype(np.float32)       # (M,)

    sq = B2 * B2
    nrm = sq.sum(axis=1, dtype=np.float64)               # row |.|^2
    scale = (np.exp(logstd_rep.astype(np.float64)) / np.sqrt(nrm)).astype(
        np.float32
    )
    A32 = B2 * scale[:, None]                            # (M, M) prescaled
    ep8 = np.ascontiguousarray(eps[:, :, 0].T).astype(np.float16)  # (M, B)
    ep_bytes = ep8.view(np.uint8)                        # (M, 2*BATCH)

    # fp16/fp8 row split — logstd_rep pattern repeats every 128 rows, so
    # the local split is identical on every core.  Rank-based: the N_FP16
    # rows with the largest exp(logstd) stay fp16 (they set the global
    # error scale); the rest go fp8.
    ls_local = np.tile(logstd.astype(np.float64), RPC // Z)       # (1024,)
    order = np.argsort(-ls_local, kind="stable")
    idx16 = np.sort(order[:N_FP16])
    idx8 = np.sort(order[N_FP16:])
    n1, n2 = len(idx16), len(idx8)
    n2p = n2 + (n2 & 1)
    wb = 2 * n1 + n2p + EPSB
    np_cols = P8 + n2
    perm = np.concatenate([idx16, idx8])

    in_maps = []
    for c in range(NCORES):
        rows = slice(c * RPC, (c + 1) * RPC)
        Ac = A32[rows, :]
        a16 = np.ascontiguousarray(Ac[idx16, :].astype(np.float16).T)
        a8 = np.ascontiguousarray(
            np.clip(Ac[idx8, :] * C_FP8, -FP8_CLIP, FP8_CLIP).astype(F8NP).T
        )
        packed = np.zeros((KT * 128, wb), dtype=np.uint8)
        packed[:, 0:2 * n1] = a16.view(np.uint8)
        packed[:, 2 * n1:2 * n1 + n2] = a8.view(np.uint8)
        packed[:, 2 * n1 + n2p:wb] = ep_bytes
        # group GRP consecutive k-tiles: partition p of group g carries the
        # packed rows of tiles g*GRP..g*GRP+GRP-1 contiguously
        bte = np.ascontiguousarray(
            packed.reshape(KT // GRP, GRP, 128, wb)
            .transpose(0, 2, 1, 3)
            .reshape(KT // GRP * 128, GRP * wb)
        )
        mu_l = mu_rep[rows]
        mu_pack = np.zeros((1, np_cols), dtype=np.float16)
        mu_pack[0, 0:n1] = mu_l[idx16].astype(np.float16)
        mu_pack[0, P8:np_cols] = (mu_l[idx8] * np.float32(C_FP8)).astype(
            np.float16
        )
        in_maps.append({"bte": bte, "mu": mu_pack})
    return in_maps, mu_rep, logstd_rep, n1, n2, perm


def _run(mu, logstd, B, eps, batch_size, trace=False, trace_kwargs=None):
    mu = np.asarray(mu, dtype=np.float32)
    logstd = np.asarray(logstd, dtype=np.float32)
    B = np.asarray(B, dtype=np.float32)
    eps = np.asarray(eps, dtype=np.float32)
    b = int(batch_size)
    assert B.shape == (1, M, M) and eps.shape == (b, M, 1) and b == BATCH

    in_maps, mu_rep, logstd_rep, n1, n2, perm = _prep_inputs(
        mu, logstd, B, eps
    )

    nc = _get_nc(n1, n2)
    kw = {}
    if trace:
        kw = dict(trace=True, trace_cores=list(range(NCORES)))
        if trace_kwargs:
            kw.update(trace_kwargs)
    res = bass_utils.run_bass_kernel_spmd(
        nc, in_maps, core_ids=list(range(NCORES)), **kw
    )

    samples_bm = np.empty((b, M), dtype=np.float32)
    for c in range(NCORES):
        out_c = np.asarray(res.results[c]["out"], dtype=np.float32)
        samples_bm[:, c * RPC + perm] = out_c
    samples = samples_bm.reshape(b, NS, Z)
    mu_out = np.broadcast_to(mu_rep[None, :], (b, M)).reshape(b, NS, Z).copy()
    logvar = (
        np.broadcast_to(2.0 * logstd_rep[None, :], (b, M)).reshape(b, NS, Z).copy()
    )
    return (mu_out, logvar, samples), res


def kernel(mu, logstd, B, eps, batch_size):
    outs, _ = _run(mu, logstd, B, eps, batch_size, trace=False)
    return outs



# revision 12
# speedup vs baseline: 1.1382x; 1.1382x over previous
"""TRN2 Bass kernel for nn_COV_75359496176097.

reference():
    B2 = B[0]                               # (8192, 8192)
    rn = sqrt(1 / sum(B2*B2, axis=1))       # row norms
    A  = rn * B2 * exp(tile(logstd, 64))[:, None]
    samples = tile(mu,64) + einsum('mk,bk->bm', A, eps[:,:,0])
    returns (mu_out, logvar, samples), each (128, 64, 128)

Strategy: shard A by rows across 8 cores (1024 rows each, no
collectives).  The row-norm and exp(logstd) scalings are diagonal, so
they are folded into A on the host, and the device runs a pure GEMM
out[b, r] = sum_k eps[k, b] * A[r, k] at the max(HBM, PE) roofline
(~358 GB/s per core / 427ns per 128-k-tile of 1024 moving columns).
Bytes are the binding constraint, so A streams in two precision tiers:

  * the N_FP16 rows with the largest exp(logstd)  -> fp16
  * all other rows -> fp8 E3M4 (TRN FP8_EXP3, IEEE bias 3), scaled by
    a global power-of-two C so values sit in fp8's normal range.

The harness error metric is relative to the GLOBAL max |sample|, set
by the largest-exp(logstd) rows; a row whose exp(logstd) is t times
smaller contributes its ~2% fp8 row-relative error only as ~2%/t
globally.  eps stays fp16 (its error feeds every output at full
scale).  mu is added by a K=1 matmul (stationary = a length-1 column
of ones) from a tiny fp16 vector, pre-scaled by C on the fp8 columns.

Packed/psum column order is [fp8 rows | fp16 rows], so psum is a
contiguous [0, 1024) window (2 banks).  PSUM start=True clears
has_written at BANK granularity (512 fp32 cols): the fp16 segment
shares bank 1 with the second fp8 segment, so on the first k-tile the
fp16 matmul runs with start=False and relies on the fp8 segment's
bank clear (per-element has_written=0 -> overwrite).

Dataflow/timing decisions (all trace-measured):
  * ALL data-tile DMAs go on ONE queue (sync) so groups complete
    strictly in consumption order at the full per-group cadence.
    Spreading them over both HWDGE queues makes the SDMA engines
    round-robin between the two rings, which delivers group PAIRS at
    twice the latency and starves the PE early on.
  * The stream is throttled to PFD_G groups ahead of PE consumption.
    Unthrottled, the deep two-ring backlog slowed the warm 512-col
    matmuls from 282ns to 512ns (SBUF write-port pressure against the
    PE's moving-operand reads).
  * The PE HAM clock gate starts at 1.2GHz and only flips to 2.4GHz
    after ~3.4us of sustained matmul activity, so the tensor block
    front-loads WARM_MM dummy N=512 matmuls (on uninitialized SBUF -
    values are irrelevant) before the first data tile, overlapping
    the DMA lead-in.
  * Epilogue: two 512-col chunks.  The final k-tile's segments inc
    s_acc in emit order, the DVE converts chunk A (psum bank 0) while
    the PE finishes bank 1, and the two 128KB output DMAs go on the
    otherwise-idle scalar queue.

Each k-tile is one DMA-ed host-packed byte row:
  [fp8 A block | fp16 A block | fp16 eps block]
"""

import sys
from contextlib import ExitStack

if "/opt/trn_rl_repo" not in sys.path:
    sys.path.insert(0, "/opt/trn_rl_repo")

import ml_dtypes
import numpy as np

import concourse.bacc as bacc
import concourse.mybir as mybir
from concourse import bass_utils

Z = 128
NS = 64
M = Z * NS          # 8192
BATCH = 128
NCORES = 8
RPC = M // NCORES   # 1024 rows of A per core
KT = M // 128       # 64 k-tiles
EPSB = 2 * BATCH    # eps block bytes per tile row

N_FP16 = 8          # rows per core kept in fp16 (largest exp(logstd))
GRP = 4             # k-tiles per DMA group
PFD_G = 6           # DMA prefetch depth in groups (issue throttle)
WARM_MM = 9         # warmup matmuls of N=512 (~3.8us at the cold 1.2GHz
                    # clock): flips the PE HAM gate to 2.4GHz right as
                    # the first data group lands
C_FP8 = 32.0        # global fp8 scale (power of two; exact in fp16/fp32)
FP8_CLIP = 15.0     # e3m4 max normal is 15.5

F8NP = np.dtype(ml_dtypes.float8_e3m4)

f32 = mybir.dt.float32
f16 = mybir.dt.float16
f8 = mybir.dt.float8e3

_nc_cache = {}


def _segments(n1, n2):
    """Matmul segments (psum_a, psum_b, is_fp16) in emit order: fp8 rows
    at psum [0, n2), fp16 rows at [n2, n2+n1).  The fp16 segment must
    share its psum bank with the last fp8 segment (its tile-0 matmul
    runs start=False and relies on that segment's bank clear)."""
    assert n2 % 2 == 0 and n2 % 512 != 0 and n1 > 0
    assert (n2 + n1 - 1) // 512 == (n2 - 1) // 512  # same bank
    segs = [(a, min(a + 512, n2), False) for a in range(0, n2, 512)]
    segs.append((n2, n2 + n1, True))
    return segs


def _build(n1, n2):
    wb = n2 + 2 * n1 + EPSB       # packed bytes per tile row
    eps_off = n2 + 2 * n1
    np_cols = n2 + n1             # psum cols used
    segs = _segments(n1, n2)
    nseg = len(segs)
    # epilogue chunks: [0, 512) and [512, np_cols); chunk j is ready
    # after every segment overlapping it has inc'd s_acc (emit order)
    chunks = []
    for ca in range(0, np_cols, 512):
        cb = min(ca + 512, np_cols)
        th = max(i + 1 for i, (sa, sb, _) in enumerate(segs)
                 if sa < cb and ca < sb)
        chunks.append((ca, cb, th))

    ng = KT // GRP                # DMA groups
    gwb = GRP * wb                # bytes per partition per group

    nc = bacc.Bacc("TRN2", debug=False)

    bte_d = nc.dram_tensor("bte", (ng * 128, gwb), mybir.dt.uint8,
                           kind="ExternalInput")
    mu_d = nc.dram_tensor("mu", (1, np_cols), f16, kind="ExternalInput")
    out_d = nc.dram_tensor("out", (BATCH, RPC), f16, kind="ExternalOutput")

    with ExitStack() as ctx:
        e = ctx.enter_context
        big8 = e(nc.sbuf_tensor("big8", [128, KT * wb], mybir.dt.uint8))
        ones = e(nc.sbuf_tensor("ones", [128, 128], f16))
        wmv = e(nc.sbuf_tensor("wmv", [128, 512], f16))
        mu_sb = e(nc.sbuf_tensor("mu_sb", [1, np_cols], f16))
        out_sb = e(nc.sbuf_tensor("out_sb", [128, RPC], f16))
        acc = e(nc.psum_tensor([128, 1024], f32))
        warm_ps = e(nc.psum_tensor([128, 512], f32))

        # one completion sem per DMA group: sem == 16 requires every one of
        # the 16 SDMA engines to have retired THIS group's descriptors
        s_t = [e(nc.semaphore(name=f"s_t{g}")) for g in range(ng)]
        s_cst = e(nc.semaphore(name="s_cst"))
        s_wm = e(nc.semaphore(name="s_wm"))
        s_pe = e(nc.semaphore(name="s_pe"))
        s_acc = e(nc.semaphore(name="s_acc"))
        s_out = e(nc.semaphore(name="s_out"))
        s_od = e(nc.semaphore(name="s_od"))

        block = e(nc.Block())

        def rhs_ap(t, sa, sb, is16):
            if is16:
                off = t * wb + n2 + 2 * (sa - n2)
                return big8[:, off:off + 2 * (sb - sa)].bitcast(f16)
            return big8[:, t * wb + sa:t * wb + sb].bitcast(f8)

        @block.sync
        def _(sync):
            for g in range(ng):
                # pace the stream to PE progress: bounds the SDMA backlog
                # (which would otherwise contend with PE SBUF reads) and
                # keeps HBM arbitration fair across the 8 cores
                if g >= PFD_G:
                    sync.wait_ge(s_pe, g - PFD_G + 1)
                sync.dma_start(
                    big8[:, g * gwb:(g + 1) * gwb],
                    bte_d.ap()[g * 128:(g + 1) * 128, :],
                ).then_inc(s_t[g], 16)

        @block.scalar
        def _(scalar):
            scalar.dma_start(mu_sb[:], mu_d.ap()[:, :]).then_inc(s_cst, 16)
            for j, (ca, cb, _) in enumerate(chunks):
                scalar.wait_ge(s_out, j + 1)
                scalar.dma_start(
                    out_d.ap()[:, ca:cb], out_sb[:, ca:cb]
                ).then_inc(s_od, 16)
            scalar.wait_ge(s_od, 16 * len(chunks))
            scalar.nop()

        @block.tensor
        def _(tensor):
            # warmup on uninitialized SBUF - no wait, starts the HAM
            # clock ramp at the earliest possible instant
            for _ in range(WARM_MM):
                nc.tensor.matmul(
                    warm_ps[:, 0:512], ones[:], wmv[:], start=True, stop=True
                )
            tensor.wait_ge(s_wm, 1)
            for g in range(ng):
                tensor.wait_ge(s_t[g], 16)
                for t in range(g * GRP, (g + 1) * GRP):
                    st, sp = t == 0, t == KT - 1
                    eps_v = big8[:, t * wb + eps_off:(t + 1) * wb].bitcast(f16)
                    for si, (sa, sb, is16) in enumerate(segs):
                        # the fp16 segment shares psum bank 1: its tile-0
                        # matmul must NOT re-clear the bank (would wipe
                        # the fp8 segment's has_written bits)
                        ins = nc.tensor.matmul(
                            acc[:, sa:sb], eps_v, rhs_ap(t, sa, sb, is16),
                            start=st and not is16, stop=sp,
                        )
                        if sp:
                            ins.then_inc(s_acc, 1)
                        elif si == nseg - 1 and t == (g + 1) * GRP - 1:
                            ins.then_inc(s_pe, 1)
                    if st:
                        # mu via K=1 matmul: out[b, r] += 1 * mu[r].  Order
                        # within a psum accumulation group doesn't matter,
                        # so run it early (off the critical tail).
                        tensor.wait_ge(s_cst, 16)
                        for sa, sb, _ in segs:
                            nc.tensor.matmul(
                                acc[:, sa:sb], ones[0:1, 0:128],
                                mu_sb[0:1, sa:sb], start=False, stop=False,
                            )

        @block.vector
        def _(vector):
            nc.vector.memset(ones[:], 1.0)
            nc.vector.memset(wmv[:], 1.0).then_inc(s_wm, 1)
            for ca, cb, th in chunks:
                vector.wait_ge(s_acc, th)
                fa, fb = max(ca, n2), min(cb, np_cols)   # fp16 part
                ea, eb = ca, min(cb, n2)                 # fp8 part
                ins = None
                if ea < eb:
                    ins = nc.vector.tensor_scalar_mul(
                        out_sb[:, ea:eb], acc[:, ea:eb], 1.0 / C_FP8
                    )
                if fa < fb:
                    ins = nc.vector.tensor_copy(
                        out_sb[:, fa:fb], acc[:, fa:fb]
                    )
                ins.then_inc(s_out, 1)

    nc.compile()
    return nc


def _get_nc(n1, n2):
    key = (n1, n2)
    if key not in _nc_cache:
        _nc_cache[key] = _build(n1, n2)
    return _nc_cache[key]


def _prep_inputs(mu, logstd, B, eps):
    B2 = B[0]                                            # (M, M) fp32
    logstd_rep = np.tile(logstd, NS).astype(np.float32)  # (M,)
    mu_rep = np.tile(mu[0], NS).astype(np.float32)       # (M,)

    sq = B2 * B2
    nrm = sq.sum(axis=1, dtype=np.float64)               # row |.|^2
    scale = (np.exp(logstd_rep.astype(np.float64)) / np.sqrt(nrm)).astype(
        np.float32
    )
    A32 = B2 * scale[:, None]                            # (M, M) prescaled
    ep8 = np.ascontiguousarray(eps[:, :, 0].T).astype(np.float16)  # (M, B)
    ep_bytes = ep8.view(np.uint8)                        # (M, 2*BATCH)

    # fp16/fp8 row split — logstd_rep pattern repeats every 128 rows, so
    # the local split is identical on every core.  Rank-based: the N_FP16
    # rows with the largest exp(logstd) stay fp16 (they set the global
    # error scale); the rest go fp8.
    ls_local = np.tile(logstd.astype(np.float64), RPC // Z)       # (1024,)
    order = np.argsort(-ls_local, kind="stable")
    idx16 = np.sort(order[:N_FP16])
    idx8 = np.sort(order[N_FP16:])
    n1, n2 = len(idx16), len(idx8)
    wb = n2 + 2 * n1 + EPSB
    np_cols = n2 + n1
    perm = np.concatenate([idx8, idx16])

    in_maps = []
    for c in range(NCORES):
        rows = slice(c * RPC, (c + 1) * RPC)
        Ac = A32[rows, :]
        a16 = np.ascontiguousarray(Ac[idx16, :].astype(np.float16).T)
        a8 = np.ascontiguousarray(
            np.clip(Ac[idx8, :] * C_FP8, -FP8_CLIP, FP8_CLIP).astype(F8NP).T
        )
        packed = np.zeros((KT * 128, wb), dtype=np.uint8)
        packed[:, 0:n2] = a8.view(np.uint8)
        packed[:, n2:n2 + 2 * n1] = a16.view(np.uint8)
        packed[:, n2 + 2 * n1:wb] = ep_bytes
        # group GRP consecutive k-tiles: partition p of group g carries the
        # packed rows of tiles g*GRP..g*GRP+GRP-1 contiguously
        bte = np.ascontiguousarray(
            packed.reshape(KT // GRP, GRP, 128, wb)
            .transpose(0, 2, 1, 3)
            .reshape(KT // GRP * 128, GRP * wb)
        )
        mu_l = mu_rep[rows]
        mu_pack = np.zeros((1, np_cols), dtype=np.float16)
        mu_pack[0, 0:n2] = (mu_l[idx8] * np.float32(C_FP8)).astype(
            np.float16
        )
        mu_pack[0, n2:np_cols] = mu_l[idx16].astype(np.float16)
        in_maps.append({"bte": bte, "mu": mu_pack})
    return in_maps, mu_rep, logstd_rep, n1, n2, perm


def _run(mu, logstd, B, eps, batch_size, trace=False, trace_kwargs=None):
    mu = np.asarray(mu, dtype=np.float32)
    logstd = np.asarray(logstd, dtype=np.float32)
    B = np.asarray(B, dtype=np.float32)
    eps = np.asarray(eps, dtype=np.float32)
    b = int(batch_size)
    assert B.shape == (1, M, M) and eps.shape == (b, M, 1) and b == BATCH

    in_maps, mu_rep, logstd_rep, n1, n2, perm = _prep_inputs(
        mu, logstd, B, eps
    )

    nc = _get_nc(n1, n2)
    kw = {}
    if trace:
        kw = dict(trace=True, trace_cores=list(range(NCORES)))
        if trace_kwargs:
            kw.update(trace_kwargs)
    res = bass_utils.run_bass_kernel_spmd(
        nc, in_maps, core_ids=list(range(NCORES)), **kw
    )

    samples_bm = np.empty((b, M), dtype=np.float32)
    for c in range(NCORES):
        out_c = np.asarray(res.results[c]["out"], dtype=np.float32)
        samples_bm[:, c * RPC + perm] = out_c
    samples = samples_bm.reshape(b, NS, Z)
    mu_out = np.broadcast_to(mu_rep[None, :], (b, M)).reshape(b, NS, Z).copy()
    logvar = (
        np.broadcast_to(2.0 * logstd_rep[None, :], (b, M)).reshape(b, NS, Z).copy()
    )
    return (mu_out, logvar, samples), res


def kernel(mu, logstd, B, eps, batch_size):
    outs, _ = _run(mu, logstd, B, eps, batch_size, trace=False)
    return outs


# revision 13
# speedup vs baseline: 1.2821x; 1.1265x over previous
"""TRN2 Bass kernel for nn_COV_75359496176097.

reference():
    B2 = B[0]                               # (8192, 8192)
    rn = sqrt(1 / sum(B2*B2, axis=1))       # row norms
    A  = rn * B2 * exp(tile(logstd, 64))[:, None]
    samples = tile(mu,64) + einsum('mk,bk->bm', A, eps[:,:,0])
    returns (mu_out, logvar, samples), each (128, 64, 128)

Strategy: shard A by rows across 8 cores (1024 rows each, no
collectives).  The row-norm and exp(logstd) scalings are diagonal, so
they are folded into A on the host, and the device runs a pure GEMM
out[b, r] = sum_k eps[k, b] * A[r, k] at the HBM roofline (~358 GB/s
per core).  Bytes are the binding constraint, so A streams in two
precision tiers:

  * the N_FP16 rows with the largest exp(logstd)  -> fp16
  * all other rows -> fp8 E4M3 (TRN FP8_EXP4: bias 7, max 240),
    scaled by a global power-of-two C.

The harness error metric is relative to the GLOBAL max |sample|, set
by the largest-exp(logstd) rows; a row whose exp(logstd) is t times
smaller contributes its fp8 row-relative error only as ~x%/t
globally.  eps is also E4M3 (required for DoubleRow).  mu is added by
a K=1 matmul from a tiny fp16 vector, pre-scaled by C on the fp8
columns.

The fp8 GEMM runs in MatmulPerfMode.DoubleRow: k-tiles are processed
in PAIRS (contraction 256 per pass, 2 fp8 MACs per PE cell per
cycle), which halves the PE streaming time and moves the kernel from
the PE/HBM ridge into a cleanly HBM-bound regime.  The fp16 rows ride
along as two plain N=8 matmuls per pair (stationary = the same e4m3
eps k-slices).

Packed/psum column order is [fp8 rows | fp16 rows], so psum is a
contiguous [0, 1024) window (2 banks).  PSUM start=True clears
has_written at BANK granularity (512 fp32 cols): the fp16 segment
shares bank 1 with the second fp8 segment, so on the first pair the
fp16 matmuls run with start=False and rely on the fp8 segment's bank
clear (per-element has_written=0 -> overwrite).

Dataflow/timing decisions (all trace-measured):
  * ALL data DMAs go on ONE queue (sync) so groups complete strictly
    in consumption order at the full per-group cadence.  Spreading
    them over both HWDGE queues makes the SDMA engines round-robin
    between the two rings, which delivers group PAIRS at twice the
    latency and starves the PE early on.
  * The stream is throttled to PFD_G groups ahead of PE consumption.
    Unthrottled, the deep two-ring backlog slowed the warm 512-col
    matmuls from 282ns to 512ns (SBUF write-port pressure against the
    PE's moving-operand reads).
  * The PE HAM clock gate starts at 1.2GHz and only flips to 2.4GHz
    after ~3.4us of sustained matmul activity, so the tensor block
    front-loads WARM_MM dummy N=512 matmuls (on uninitialized SBUF -
    values are irrelevant) before the first data tile, overlapping
    the DMA lead-in.
  * Epilogue: two 512-col chunks.  The final pair's matmuls inc
    s_acc in emit order, the DVE converts chunk A (psum bank 0) while
    the PE finishes bank 1; chunk A's 128KB output DMA goes on the
    (by then idle) sync queue and chunk B's on the scalar queue so
    the two issues overlap.

Each k-tile PAIR is one host-packed byte row (per partition k):
  [A8_t0 | A8_t1 | A16_t0 | A16_t1 | eps8_t0 | eps8_t1]
"""

import sys
from contextlib import ExitStack

if "/opt/trn_rl_repo" not in sys.path:
    sys.path.insert(0, "/opt/trn_rl_repo")

import ml_dtypes
import numpy as np

import concourse.bacc as bacc
import concourse.mybir as mybir
from concourse import bass_utils

Z = 128
NS = 64
M = Z * NS          # 8192
BATCH = 128
NCORES = 8
RPC = M // NCORES   # 1024 rows of A per core
KT = M // 128       # 64 k-tiles
NP = KT // 2        # 32 k-tile pairs (DoubleRow processes K=256 per pass)

N_FP16 = 8          # rows per core kept in fp16 (largest exp(logstd))
GRP = 2             # k-tile PAIRS per DMA group (4 k-tiles, ~580KB)
PFD_G = 6           # DMA prefetch depth in groups (issue throttle)
WARM_MM = 9         # warmup matmuls of N=512 (~3.8us at the cold 1.2GHz
                    # clock): flips the PE HAM gate to 2.4GHz right as
                    # the first data group lands
C_FP8 = 32.0        # global fp8 scale (power of two; exact in fp16/fp32)
FP8_CLIP = 240.0    # e4m3 max normal (TRN FP8_EXP4 and IEEE e4m3 agree)

E4NP = np.dtype(ml_dtypes.float8_e4m3)   # IEEE-style e4m3: bias 7, max 240

f32 = mybir.dt.float32
f16 = mybir.dt.float16
f8e4 = mybir.dt.float8e4
DR = mybir.MatmulPerfMode.DoubleRow

_nc_cache = {}


def _build(n1, n2):
    # per-partition byte layout of one k-tile pair
    a8b = 2 * n2            # two fp8 A blocks
    a16o = a8b              # two fp16 A blocks (2*n1 bytes each)
    epso = a8b + 4 * n1     # two e4m3 eps blocks (128 bytes each)
    pwb = epso + 2 * 128
    np_cols = n2 + n1       # psum cols used
    assert n2 % 2 == 0 and 512 < np_cols <= 1024

    ng = NP // GRP          # DMA groups
    gwb = GRP * pwb         # bytes per partition per group

    nc = bacc.Bacc("TRN2", debug=False)

    bte_d = nc.dram_tensor("bte", (ng * 128, gwb), mybir.dt.uint8,
                           kind="ExternalInput")
    mu_d = nc.dram_tensor("mu", (1, np_cols), f16, kind="ExternalInput")
    out_d = nc.dram_tensor("out", (BATCH, RPC), f16, kind="ExternalOutput")

    with ExitStack() as ctx:
        e = ctx.enter_context
        big8 = e(nc.sbuf_tensor("big8", [128, NP * pwb], mybir.dt.uint8))
        ones = e(nc.sbuf_tensor("ones", [128, 128], f16))
        wmv = e(nc.sbuf_tensor("wmv", [128, 512], f16))
        mu_sb = e(nc.sbuf_tensor("mu_sb", [1, np_cols], f16))
        out_sb = e(nc.sbuf_tensor("out_sb", [128, RPC], f16))
        acc = e(nc.psum_tensor([128, 1024], f32))
        warm_ps = e(nc.psum_tensor([128, 512], f32))

        # one completion sem per DMA group: sem == 16 requires every one of
        # the 16 SDMA engines to have retired THIS group's descriptors
        s_t = [e(nc.semaphore(name=f"s_t{g}")) for g in range(ng)]
        s_cst = e(nc.semaphore(name="s_cst"))
        s_wm = e(nc.semaphore(name="s_wm"))
        s_pe = e(nc.semaphore(name="s_pe"))
        s_acc = e(nc.semaphore(name="s_acc"))
        s_out = e(nc.semaphore(name="s_out"))
        s_od = e(nc.semaphore(name="s_od"))

        block = e(nc.Block())

        def pair_a8(p):
            # [128, 2, n2] e4m3: j-major blocks, strides (n2, 1)
            base = p * pwb
            return (big8[:, base:base + 2 * n2].bitcast(f8e4)
                    .rearrange("p (j n) -> p j n", j=2))

        def pair_eps(p):
            # [128, 2, 128] e4m3 stationary for DoubleRow (K=256)
            base = p * pwb + epso
            return (big8[:, base:base + 256].bitcast(f8e4)
                    .rearrange("p (j n) -> p j n", j=2))

        def eps_j(p, j):
            base = p * pwb + epso + 128 * j
            return big8[:, base:base + 128].bitcast(f8e4)

        def a16_j(p, j):
            base = p * pwb + a16o + 2 * n1 * j
            return big8[:, base:base + 2 * n1].bitcast(f16)

        @block.sync
        def _(sync):
            for g in range(ng):
                # pace the stream to PE progress: bounds the SDMA backlog
                # (which would otherwise contend with PE SBUF reads) and
                # keeps HBM arbitration fair across the 8 cores
                if g >= PFD_G:
                    sync.wait_ge(s_pe, g - PFD_G + 1)
                sync.dma_start(
                    big8[:, g * gwb:(g + 1) * gwb],
                    bte_d.ap()[g * 128:(g + 1) * 128, :],
                ).then_inc(s_t[g], 16)
            # chunk A output: the sync queue is idle by the time the
            # epilogue runs, so the two output DMAs issue concurrently
            sync.wait_ge(s_out, 1)
            sync.dma_start(out_d.ap()[:, 0:512], out_sb[:, 0:512]).then_inc(
                s_od, 16
            )

        @block.scalar
        def _(scalar):
            scalar.dma_start(mu_sb[:], mu_d.ap()[:, :]).then_inc(s_cst, 16)
            scalar.wait_ge(s_out, 2)
            scalar.dma_start(
                out_d.ap()[:, 512:np_cols], out_sb[:, 512:np_cols]
            ).then_inc(s_od, 16)
            scalar.wait_ge(s_od, 32)
            scalar.nop()

        @block.tensor
        def _(tensor):
            # warmup on uninitialized SBUF - no wait, starts the HAM
            # clock ramp at the earliest possible instant
            for _ in range(WARM_MM):
                nc.tensor.matmul(
                    warm_ps[:, 0:512], ones[:], wmv[:], start=True, stop=True
                )
            tensor.wait_ge(s_wm, 1)
            for g in range(ng):
                tensor.wait_ge(s_t[g], 16)
                for p in range(g * GRP, (g + 1) * GRP):
                    st, sp = p == 0, p == NP - 1
                    epsp = pair_eps(p)
                    a8 = pair_a8(p)
                    for si, (sa, sb) in enumerate(((0, 512), (512, n2))):
                        ins = nc.tensor.matmul(
                            acc[:, sa:sb], epsp, a8[:, :, sa:sb],
                            start=st, stop=sp, perf_mode=DR,
                        )
                        if sp:
                            ins.then_inc(s_acc, 1)
                        elif si == 1 and p == (g + 1) * GRP - 1:
                            ins.then_inc(s_pe, 1)
                    # fp16 rows: two plain matmuls (one per k-tile of the
                    # pair).  start=False always: on pair 0 they rely on
                    # the DR segment's bank-1 clear (has_written=0 ->
                    # overwrite) because they share its psum bank.
                    for j in (0, 1):
                        ins = nc.tensor.matmul(
                            acc[:, n2:np_cols], eps_j(p, j), a16_j(p, j),
                            start=False, stop=sp,
                        )
                        if sp and j == 1:
                            ins.then_inc(s_acc, 1)
                    if st:
                        # mu via K=1 matmul: out[b, r] += 1 * mu[r].  Order
                        # within a psum accumulation group doesn't matter,
                        # so run it early (off the critical tail).
                        tensor.wait_ge(s_cst, 16)
                        for sa, sb in ((0, 512), (512, np_cols)):
                            nc.tensor.matmul(
                                acc[:, sa:sb], ones[0:1, 0:128],
                                mu_sb[0:1, sa:sb], start=False, stop=False,
                            )

        @block.vector
        def _(vector):
            nc.vector.memset(ones[:], 1.0)
            nc.vector.memset(wmv[:], 1.0).then_inc(s_wm, 1)
            # chunk A: psum bank 0, ready after the final pair's first DR
            # segment (s_acc=1); chunk B: bank 1, ready after everything
            # (s_acc=3)
            vector.wait_ge(s_acc, 1)
            nc.vector.tensor_scalar_mul(
                out_sb[:, 0:512], acc[:, 0:512], 1.0 / C_FP8
            ).then_inc(s_out, 1)
            vector.wait_ge(s_acc, 3)
            nc.vector.tensor_scalar_mul(
                out_sb[:, 512:n2], acc[:, 512:n2], 1.0 / C_FP8
            )
            nc.vector.tensor_copy(
                out_sb[:, n2:np_cols], acc[:, n2:np_cols]
            ).then_inc(s_out, 1)

    nc.compile()
    return nc


def _get_nc(n1, n2):
    key = (n1, n2)
    if key not in _nc_cache:
        _nc_cache[key] = _build(n1, n2)
    return _nc_cache[key]


def _prep_inputs(mu, logstd, B, eps):
    B2 = B[0]                                            # (M, M) fp32
    logstd_rep = np.tile(logstd, NS).astype(np.float32)  # (M,)
    mu_rep = np.tile(mu[0], NS).astype(np.float32)       # (M,)

    sq = B2 * B2
    nrm = sq.sum(axis=1, dtype=np.float64)               # row |.|^2
    scale = (np.exp(logstd_rep.astype(np.float64)) / np.sqrt(nrm)).astype(
        np.float32
    )
    A32 = B2 * scale[:, None]                            # (M, M) prescaled
    epsT = np.ascontiguousarray(eps[:, :, 0].T)          # (M, B) fp32
    ep8 = np.clip(epsT, -FP8_CLIP, FP8_CLIP).astype(E4NP)
    ep_bytes = ep8.view(np.uint8)                        # (M, BATCH)

    # fp16/fp8 row split — logstd_rep pattern repeats every 128 rows, so
    # the local split is identical on every core.  Rank-based: the N_FP16
    # rows with the largest exp(logstd) stay fp16 (they set the global
    # error scale); the rest go fp8.
    ls_local = np.tile(logstd.astype(np.float64), RPC // Z)       # (1024,)
    order = np.argsort(-ls_local, kind="stable")
    idx16 = np.sort(order[:N_FP16])
    idx8 = np.sort(order[N_FP16:])
    n1, n2 = len(idx16), len(idx8)
    np_cols = n2 + n1
    pwb = 2 * n2 + 4 * n1 + 256
    perm = np.concatenate([idx8, idx16])

    def pair_blocks(x):
        # (KT*128, w) per-tile rows -> (NP*128, 2*w): partition k of pair
        # p carries tile 2p's row then tile 2p+1's row
        w = x.shape[1]
        return (x.reshape(NP, 2, 128, w).transpose(0, 2, 1, 3)
                .reshape(NP * 128, 2 * w))

    ep_pair = pair_blocks(ep_bytes)                      # (NP*128, 256)

    in_maps = []
    for c in range(NCORES):
        rows = slice(c * RPC, (c + 1) * RPC)
        Ac = A32[rows, :]
        a16 = np.ascontiguousarray(Ac[idx16, :].astype(np.float16).T)
        a8 = np.ascontiguousarray(
            np.clip(Ac[idx8, :] * C_FP8, -FP8_CLIP, FP8_CLIP).astype(E4NP).T
        )
        packed = np.concatenate(
            [pair_blocks(a8.view(np.uint8)),
             pair_blocks(a16.view(np.uint8)),
             ep_pair],
            axis=1,
        )
        assert packed.shape == (NP * 128, pwb)
        # group GRP consecutive pairs: partition k of group g carries the
        # packed rows of pairs g*GRP..g*GRP+GRP-1 contiguously
        bte = np.ascontiguousarray(
            packed.reshape(NP // GRP, GRP, 128, pwb)
            .transpose(0, 2, 1, 3)
            .reshape(NP // GRP * 128, GRP * pwb)
        )
        mu_l = mu_rep[rows]
        mu_pack = np.zeros((1, np_cols), dtype=np.float16)
        mu_pack[0, 0:n2] = (mu_l[idx8] * np.float32(C_FP8)).astype(
            np.float16
        )
        mu_pack[0, n2:np_cols] = mu_l[idx16].astype(np.float16)
        in_maps.append({"bte": bte, "mu": mu_pack})
    return in_maps, mu_rep, logstd_rep, n1, n2, perm


def _run(mu, logstd, B, eps, batch_size, trace=False, trace_kwargs=None):
    mu = np.asarray(mu, dtype=np.float32)
    logstd = np.asarray(logstd, dtype=np.float32)
    B = np.asarray(B, dtype=np.float32)
    eps = np.asarray(eps, dtype=np.float32)
    b = int(batch_size)
    assert B.shape == (1, M, M) and eps.shape == (b, M, 1) and b == BATCH

    in_maps, mu_rep, logstd_rep, n1, n2, perm = _prep_inputs(
        mu, logstd, B, eps
    )

    nc = _get_nc(n1, n2)
    kw = {}
    if trace:
        kw = dict(trace=True, trace_cores=list(range(NCORES)))
        if trace_kwargs:
            kw.update(trace_kwargs)
    res = bass_utils.run_bass_kernel_spmd(
        nc, in_maps, core_ids=list(range(NCORES)), **kw
    )

    samples_bm = np.empty((b, M), dtype=np.float32)
    for c in range(NCORES):
        out_c = np.asarray(res.results[c]["out"], dtype=np.float32)
        samples_bm[:, c * RPC + perm] = out_c
    samples = samples_bm.reshape(b, NS, Z)
    mu_out = np.broadcast_to(mu_rep[None, :], (b, M)).reshape(b, NS, Z).copy()
    logvar = (
        np.broadcast_to(2.0 * logstd_rep[None, :], (b, M)).reshape(b, NS, Z).copy()
    )
    return (mu_out, logvar, samples), res


def kernel(mu, logstd, B, eps, batch_size):
    outs, _ = _run(mu, logstd, B, eps, batch_size, trace=False)
    return outs


# revision 15
# speedup vs baseline: 1.3254x; 1.0338x over previous
"""TRN2 Bass kernel for nn_COV_75359496176097.

reference():
    B2 = B[0]                               # (8192, 8192)
    rn = sqrt(1 / sum(B2*B2, axis=1))       # row norms
    A  = rn * B2 * exp(tile(logstd, 64))[:, None]
    samples = tile(mu,64) + einsum('mk,bk->bm', A, eps[:,:,0])
    returns (mu_out, logvar, samples), each (128, 64, 128)

Strategy: shard A by rows across 8 cores (1024 rows each, no
collectives).  The row-norm and exp(logstd) scalings are diagonal, so
they are folded into A on the host, and the device runs a pure GEMM
out[b, r] = sum_k eps[k, b] * A[r, k] at the HBM roofline (~358 GB/s
per core).  Bytes are the binding constraint, so A streams in two
precision tiers:

  * the N_FP16 rows with the largest exp(logstd)  -> fp16
  * all other rows -> fp8 E4M3 (TRN FP8_EXP4: bias 7, max 240),
    scaled by a global power-of-two C.

The harness error metric is relative to the GLOBAL max |sample|, set
by the largest-exp(logstd) rows; a row whose exp(logstd) is t times
smaller contributes its fp8 row-relative error only as ~x%/t
globally.  eps is also E4M3 (required for DoubleRow).  mu is added by
a K=1 matmul from a tiny fp16 vector, pre-scaled by C on the fp8
columns.

The fp8 GEMM runs in MatmulPerfMode.DoubleRow: k-tiles are processed
in PAIRS (contraction 256 per pass, 2 fp8 MACs per PE cell per
cycle), which halves the PE streaming time and moves the kernel from
the PE/HBM ridge into a cleanly HBM-bound regime.  The fp16 rows ride
along as two plain N=8 matmuls per pair (stationary = the same e4m3
eps k-slices).

Packed/psum column order is [fp8 rows | fp16 rows], so psum is a
contiguous [0, 1024) window (2 banks).  PSUM start=True clears
has_written at BANK granularity (512 fp32 cols): the fp16 segment
shares bank 1 with the second fp8 segment, so on the first pair the
fp16 matmuls run with start=False and rely on the fp8 segment's bank
clear (per-element has_written=0 -> overwrite).

Dataflow/timing decisions (all trace-measured):
  * ALL data DMAs go on ONE queue (sync) so groups complete strictly
    in consumption order at the full per-group cadence.  Spreading
    them over both HWDGE queues makes the SDMA engines round-robin
    between the two rings, which delivers group PAIRS at twice the
    latency and starves the PE early on.
  * The stream is throttled to PFD_G groups ahead of PE consumption.
    Unthrottled, the deep two-ring backlog slowed the warm 512-col
    matmuls from 282ns to 512ns (SBUF write-port pressure against the
    PE's moving-operand reads).
  * The PE HAM clock gate starts at 1.2GHz and only flips to 2.4GHz
    after ~3.4us of sustained matmul activity, so the tensor block
    front-loads WARM_MM dummy N=512 matmuls (on uninitialized SBUF -
    values are irrelevant) before the first data tile, overlapping
    the DMA lead-in.
  * Epilogue: two 512-col chunks.  The final pair's matmuls inc
    s_acc in emit order, the DVE converts chunk A (psum bank 0) while
    the PE finishes bank 1; chunk A's 128KB output DMA goes on the
    (by then idle) sync queue and chunk B's on the scalar queue so
    the two issues overlap.

Each k-tile PAIR is one host-packed byte row (per partition k):
  [A8_t0 | A8_t1 | A16_t0 | A16_t1 | eps8_t0 | eps8_t1]
"""

import sys
from contextlib import ExitStack

if "/opt/trn_rl_repo" not in sys.path:
    sys.path.insert(0, "/opt/trn_rl_repo")

import ml_dtypes
import numpy as np

import concourse.bacc as bacc
import concourse.mybir as mybir
from concourse import bass_utils

Z = 128
NS = 64
M = Z * NS          # 8192
BATCH = 128
NCORES = 8
RPC = M // NCORES   # 1024 rows of A per core
KT = M // 128       # 64 k-tiles
NP = KT // 2        # 32 k-tile pairs (DoubleRow processes K=256 per pass)

N_FP16 = 8          # rows per core kept in fp16 (largest exp(logstd))
GRP = 2             # k-tile PAIRS per DMA group (4 k-tiles, ~580KB)
PFD_G = 6           # DMA prefetch depth in groups (issue throttle)
WARM_MM = 9         # warmup matmuls of N=512 (~3.8us at the cold 1.2GHz
                    # clock): flips the PE HAM gate to 2.4GHz right as
                    # the first data group lands
DUMMY_MM = 2        # filler matmuls per DMA group: with DoubleRow the PE
                    # outruns the DMA stream and would micro-idle between
                    # groups, which re-throttles the HAM clock gate to
                    # 1.2GHz - at which point a pair is SLOWER than the
                    # group cadence and stalls compound.  Two dummy N=512
                    # matmuls absorb the idle (PE has the slack) and keep
                    # the gate at 2.4GHz.
C_FP8 = 32.0        # global fp8 scale (power of two; exact in fp16/fp32)
FP8_CLIP = 240.0    # e4m3 max normal (TRN FP8_EXP4 and IEEE e4m3 agree)

E4NP = np.dtype(ml_dtypes.float8_e4m3)   # IEEE-style e4m3: bias 7, max 240

f32 = mybir.dt.float32
f16 = mybir.dt.float16
f8e4 = mybir.dt.float8e4
DR = mybir.MatmulPerfMode.DoubleRow

_nc_cache = {}


def _build(n1, n2):
    # per-partition byte layout of one k-tile pair
    a8b = 2 * n2            # two fp8 A blocks
    a16o = a8b              # two fp16 A blocks (2*n1 bytes each)
    epso = a8b + 4 * n1     # two e4m3 eps blocks (128 bytes each)
    pwb = epso + 2 * 128
    np_cols = n2 + n1       # psum cols used
    assert n2 % 2 == 0 and 512 < np_cols <= 1024

    ng = NP // GRP          # DMA groups
    gwb = GRP * pwb         # bytes per partition per group

    nc = bacc.Bacc("TRN2", debug=False)

    bte_d = nc.dram_tensor("bte", (ng * 128, gwb), mybir.dt.uint8,
                           kind="ExternalInput")
    mu_d = nc.dram_tensor("mu", (1, np_cols), f16, kind="ExternalInput")
    out_d = nc.dram_tensor("out", (BATCH, RPC), f16, kind="ExternalOutput")

    with ExitStack() as ctx:
        e = ctx.enter_context
        big8 = e(nc.sbuf_tensor("big8", [128, NP * pwb], mybir.dt.uint8))
        ones = e(nc.sbuf_tensor("ones", [128, 128], f16))
        wmv = e(nc.sbuf_tensor("wmv", [128, 512], f16))
        mu_sb = e(nc.sbuf_tensor("mu_sb", [1, np_cols], f16))
        out_sb = e(nc.sbuf_tensor("out_sb", [128, RPC], f16))
        acc = e(nc.psum_tensor([128, 1024], f32))
        warm_ps = e(nc.psum_tensor([128, 512], f32))

        # one completion sem per DMA group: sem == 16 requires every one of
        # the 16 SDMA engines to have retired THIS group's descriptors
        s_t = [e(nc.semaphore(name=f"s_t{g}")) for g in range(ng)]
        s_cst = e(nc.semaphore(name="s_cst"))
        s_wm = e(nc.semaphore(name="s_wm"))
        s_pe = e(nc.semaphore(name="s_pe"))
        s_acc = e(nc.semaphore(name="s_acc"))
        s_out = e(nc.semaphore(name="s_out"))
        s_od = e(nc.semaphore(name="s_od"))

        block = e(nc.Block())

        def pair_a8(p):
            # [128, 2, n2] e4m3: j-major blocks, strides (n2, 1)
            base = p * pwb
            return (big8[:, base:base + 2 * n2].bitcast(f8e4)
                    .rearrange("p (j n) -> p j n", j=2))

        def pair_eps(p):
            # [128, 2, 128] e4m3 stationary for DoubleRow (K=256)
            base = p * pwb + epso
            return (big8[:, base:base + 256].bitcast(f8e4)
                    .rearrange("p (j n) -> p j n", j=2))

        def eps_j(p, j):
            base = p * pwb + epso + 128 * j
            return big8[:, base:base + 128].bitcast(f8e4)

        def a16_j(p, j):
            base = p * pwb + a16o + 2 * n1 * j
            return big8[:, base:base + 2 * n1].bitcast(f16)

        @block.sync
        def _(sync):
            for g in range(ng):
                # pace the stream to PE progress: bounds the SDMA backlog
                # (which would otherwise contend with PE SBUF reads) and
                # keeps HBM arbitration fair across the 8 cores
                if g >= PFD_G:
                    sync.wait_ge(s_pe, g - PFD_G + 1)
                sync.dma_start(
                    big8[:, g * gwb:(g + 1) * gwb],
                    bte_d.ap()[g * 128:(g + 1) * 128, :],
                ).then_inc(s_t[g], 16)
            # chunk A output: the sync queue is idle by the time the
            # epilogue runs, so the two output DMAs issue concurrently
            sync.wait_ge(s_out, 1)
            sync.dma_start(out_d.ap()[:, 0:512], out_sb[:, 0:512]).then_inc(
                s_od, 16
            )

        @block.scalar
        def _(scalar):
            scalar.dma_start(mu_sb[:], mu_d.ap()[:, :]).then_inc(s_cst, 16)
            scalar.wait_ge(s_out, 2)
            scalar.dma_start(
                out_d.ap()[:, 512:np_cols], out_sb[:, 512:np_cols]
            ).then_inc(s_od, 16)
            scalar.wait_ge(s_od, 32)
            scalar.nop()

        @block.tensor
        def _(tensor):
            # warmup on uninitialized SBUF - no wait, starts the HAM
            # clock ramp at the earliest possible instant
            for _ in range(WARM_MM):
                nc.tensor.matmul(
                    warm_ps[:, 0:512], ones[:], wmv[:], start=True, stop=True
                )
            tensor.wait_ge(s_wm, 1)
            for g in range(ng):
                if 0 < g < ng - 1:
                    for _ in range(DUMMY_MM):
                        nc.tensor.matmul(
                            warm_ps[:, 0:512], ones[:], wmv[:],
                            start=True, stop=True,
                        )
                tensor.wait_ge(s_t[g], 16)
                for p in range(g * GRP, (g + 1) * GRP):
                    st, sp = p == 0, p == NP - 1
                    epsp = pair_eps(p)
                    a8 = pair_a8(p)
                    for si, (sa, sb) in enumerate(((0, 512), (512, n2))):
                        ins = nc.tensor.matmul(
                            acc[:, sa:sb], epsp, a8[:, :, sa:sb],
                            start=st, stop=sp, perf_mode=DR,
                        )
                        if sp:
                            ins.then_inc(s_acc, 1)
                        elif si == 1 and p == (g + 1) * GRP - 1:
                            ins.then_inc(s_pe, 1)
                    # fp16 rows: two plain matmuls (one per k-tile of the
                    # pair).  start=False always: on pair 0 they rely on
                    # the DR segment's bank-1 clear (has_written=0 ->
                    # overwrite) because they share its psum bank.
                    for j in (0, 1):
                        ins = nc.tensor.matmul(
                            acc[:, n2:np_cols], eps_j(p, j), a16_j(p, j),
                            start=False, stop=sp,
                        )
                        if sp and j == 1:
                            ins.then_inc(s_acc, 1)
                    if st:
                        # mu via K=1 matmul: out[b, r] += 1 * mu[r].  Order
                        # within a psum accumulation group doesn't matter,
                        # so run it early (off the critical tail).
                        tensor.wait_ge(s_cst, 16)
                        for sa, sb in ((0, 512), (512, np_cols)):
                            nc.tensor.matmul(
                                acc[:, sa:sb], ones[0:1, 0:128],
                                mu_sb[0:1, sa:sb], start=False, stop=False,
                            )

        @block.vector
        def _(vector):
            nc.vector.memset(ones[:], 1.0)
            nc.vector.memset(wmv[:], 1.0).then_inc(s_wm, 1)
            # chunk A: psum bank 0, ready after the final pair's first DR
            # segment (s_acc=1); chunk B: bank 1, ready after everything
            # (s_acc=3)
            vector.wait_ge(s_acc, 1)
            nc.vector.tensor_scalar_mul(
                out_sb[:, 0:512], acc[:, 0:512], 1.0 / C_FP8
            ).then_inc(s_out, 1)
            vector.wait_ge(s_acc, 3)
            nc.vector.tensor_scalar_mul(
                out_sb[:, 512:n2], acc[:, 512:n2], 1.0 / C_FP8
            )
            nc.vector.tensor_copy(
                out_sb[:, n2:np_cols], acc[:, n2:np_cols]
            ).then_inc(s_out, 1)

    nc.compile()
    return nc


def _get_nc(n1, n2):
    key = (n1, n2)
    if key not in _nc_cache:
        _nc_cache[key] = _build(n1, n2)
    return _nc_cache[key]


def _prep_inputs(mu, logstd, B, eps):
    B2 = B[0]                                            # (M, M) fp32
    logstd_rep = np.tile(logstd, NS).astype(np.float32)  # (M,)
    mu_rep = np.tile(mu[0], NS).astype(np.float32)       # (M,)

    sq = B2 * B2
    nrm = sq.sum(axis=1, dtype=np.float64)               # row |.|^2
    scale = (np.exp(logstd_rep.astype(np.float64)) / np.sqrt(nrm)).astype(
        np.float32
    )
    A32 = B2 * scale[:, None]                            # (M, M) prescaled
    epsT = np.ascontiguousarray(eps[:, :, 0].T)          # (M, B) fp32
    ep8 = np.clip(epsT, -FP8_CLIP, FP8_CLIP).astype(E4NP)
    ep_bytes = ep8.view(np.uint8)                        # (M, BATCH)

    # fp16/fp8 row split — logstd_rep pattern repeats every 128 rows, so
    # the local split is identical on every core.  Rank-based: the N_FP16
    # rows with the largest exp(logstd) stay fp16 (they set the global
    # error scale); the rest go fp8.
    ls_local = np.tile(logstd.astype(np.float64), RPC // Z)       # (1024,)
    order = np.argsort(-ls_local, kind="stable")
    idx16 = np.sort(order[:N_FP16])
    idx8 = np.sort(order[N_FP16:])
    n1, n2 = len(idx16), len(idx8)
    np_cols = n2 + n1
    pwb = 2 * n2 + 4 * n1 + 256
    perm = np.concatenate([idx8, idx16])

    def pair_blocks(x):
        # (KT*128, w) per-tile rows -> (NP*128, 2*w): partition k of pair
        # p carries tile 2p's row then tile 2p+1's row
        w = x.shape[1]
        return (x.reshape(NP, 2, 128, w).transpose(0, 2, 1, 3)
                .reshape(NP * 128, 2 * w))

    ep_pair = pair_blocks(ep_bytes)                      # (NP*128, 256)

    in_maps = []
    for c in range(NCORES):
        rows = slice(c * RPC, (c + 1) * RPC)
        Ac = A32[rows, :]
        a16 = np.ascontiguousarray(Ac[idx16, :].astype(np.float16).T)
        a8 = np.ascontiguousarray(
            np.clip(Ac[idx8, :] * C_FP8, -FP8_CLIP, FP8_CLIP).astype(E4NP).T
        )
        packed = np.concatenate(
            [pair_blocks(a8.view(np.uint8)),
             pair_blocks(a16.view(np.uint8)),
             ep_pair],
            axis=1,
        )
        assert packed.shape == (NP * 128, pwb)
        # group GRP consecutive pairs: partition k of group g carries the
        # packed rows of pairs g*GRP..g*GRP+GRP-1 contiguously
        bte = np.ascontiguousarray(
            packed.reshape(NP // GRP, GRP, 128, pwb)
            .transpose(0, 2, 1, 3)
            .reshape(NP // GRP * 128, GRP * pwb)
        )
        mu_l = mu_rep[rows]
        mu_pack = np.zeros((1, np_cols), dtype=np.float16)
        mu_pack[0, 0:n2] = (mu_l[idx8] * np.float32(C_FP8)).astype(
            np.float16
        )
        mu_pack[0, n2:np_cols] = mu_l[idx16].astype(np.float16)
        in_maps.append({"bte": bte, "mu": mu_pack})
    return in_maps, mu_rep, logstd_rep, n1, n2, perm


def _run(mu, logstd, B, eps, batch_size, trace=False, trace_kwargs=None):
    mu = np.asarray(mu, dtype=np.float32)
    logstd = np.asarray(logstd, dtype=np.float32)
    B = np.asarray(B, dtype=np.float32)
    eps = np.asarray(eps, dtype=np.float32)
    b = int(batch_size)
    assert B.shape == (1, M, M) and eps.shape == (b, M, 1) and b == BATCH

    in_maps, mu_rep, logstd_rep, n1, n2, perm = _prep_inputs(
        mu, logstd, B, eps
    )

    nc = _get_nc(n1, n2)
    kw = {}
    if trace:
        kw = dict(trace=True, trace_cores=list(range(NCORES)))
        if trace_kwargs:
            kw.update(trace_kwargs)
    res = bass_utils.run_bass_kernel_spmd(
        nc, in_maps, core_ids=list(range(NCORES)), **kw
    )

    samples_bm = np.empty((b, M), dtype=np.float32)
    for c in range(NCORES):
        out_c = np.asarray(res.results[c]["out"], dtype=np.float32)
        samples_bm[:, c * RPC + perm] = out_c
    samples = samples_bm.reshape(b, NS, Z)
    mu_out = np.broadcast_to(mu_rep[None, :], (b, M)).reshape(b, NS, Z).copy()
    logvar = (
        np.broadcast_to(2.0 * logstd_rep[None, :], (b, M)).reshape(b, NS, Z).copy()
    )
    return (mu_out, logvar, samples), res


def kernel(mu, logstd, B, eps, batch_size):
    outs, _ = _run(mu, logstd, B, eps, batch_size, trace=False)
    return outs
